# revision 15
# baseline (speedup 1.0000x reference)
"""Trainium2 Bass kernel for NeuralDecisionTree (soft decision tree MoE).

Strategy: data-parallel over batch across 8 NeuronCores (1024 rows/core),
weights replicated, all matmul operands in bfloat16 (fp32 PSUM accumulate).
bf16 (vs the earlier fp32r version) lets LDWEIGHTS pipeline ahead of the
streaming matmuls (fp32r fuses the 4-byte weight load serially into every
matmul: ~60ns/MM extra), halves HBM traffic, and permits PSUM writes at
non-zero partition bases, which unlocks column-tiled (tile_position)
concurrent matmuls for every stationary-width<128 GEMM.

Per-core dataflow (activations kept in [feature, batch] layout):
  z      = router_W @ x^T     16 MMs in 32-col-tile mode, packed [128, 512]
                              (t0 rows 0-63, t1 rows 64-127)
  S_t    = [ln s; ln(1-s)]    routing ACT chain on the packed z, 2 DVE builds
  log p  = A @ S -> exp       a64 (pt, 4-tile burst) + per-group afl
                              pre-broadcast selection matmuls (128-mode)
  L1:    h1[l] = relu(W1_l^T x^T + b1)  4 K-chunk MMs per (leaf, t) into a
         2-bank PSUM tile (t0|t1), one paired [128,2,512] evacuation
  L2:    per leaf pair: 4 concurrent 32-col-tile MMs -> [128,512] PSUM
  L3:    per group: 4 pairs as 4 concurrent 32-col tiles -> [128,512] pred
  mix:   prod = pred * p (DVE), out += rsel^T prod (32-mode MM bursts),
         b3 term via padded b3 stationaries against packed exp(pt).

The kernel alternates one 128-mode window (L1 + afl) and one 32-mode
window (L2 + deferred L3/prod/r of earlier groups) per 8-leaf block, so
the PE pays only 2 tiling-mode switches per block.  A run of dummy
matmuls on the (tiny, first-DMA'd) router weights warms the PE HAM clock
gate during the initial x/W1 DMA window.
"""

import sys

import numpy as np
import ml_dtypes

if "/opt/trn_rl_repo" not in sys.path:
    sys.path.insert(0, "/opt/trn_rl_repo")

import concourse.bass as bass  # noqa: F401  (import keeps parity with env)
import concourse.hw_specs as hw_specs
import concourse.tile as tile
from concourse import bacc, mybir
from concourse.bass_utils import run_bass_kernel_spmd

_ONE_TABLE = "natural_log_exp_and_others"
_orig_get_tables = hw_specs.get_activation_tables


def _patched_get_tables(module_arch):
    """Confine activation-table choice to one set that covers every ACT
    func this kernel uses (exp/ln/relu/abs/copy/identity), so the greedy
    per-instruction table picker never ping-pongs between sets."""
    tables = dict(_orig_get_tables(module_arch))
    keep = tables[_ONE_TABLE]
    return {k: (v if k == _ONE_TABLE else (v & set()) or set())
            if k != _ONE_TABLE else keep for k, v in tables.items()}


f32 = mybir.dt.float32
bf16 = mybir.dt.bfloat16
AF = mybir.ActivationFunctionType
ALU = mybir.AluOpType
BF = ml_dtypes.bfloat16

# Problem shape (hardcoded; harness contract)
B = 8192
D = 512
H1 = 128
H2 = 64
OUT = 8
L = 64
NI = 63
NCORES = 8
BC = B // NCORES        # 1024 rows per core
N = 512                 # batch tile (matmul free dim / PSUM bank)
T = BC // N             # 2 batch tiles per core
KC = D // 128           # 4 contraction chunks for the input dim
NPAIR = L // 2          # 32 leaf pairs
NG = 8                  # 8-leaf groups


def _leaf_path_rows(leaf):
    """Rows of the [128] log-sigmoid stack contributing to log p(leaf).

    Row n (n<63) holds ln d_n; row 64+n holds ln(1-d_n); rows 63 and 127
    are zero pads.  Mirrors the reference's level-wise p interleave.
    """
    rows = []
    for k in range(6):
        prefix = leaf >> (6 - k)
        node = (2 ** k - 1) + prefix
        bit = (leaf >> (5 - k)) & 1
        rows.append(node + 64 * bit)
    return rows


def build_nc():
    nc = bacc.Bacc("TRN2", target_bir_lowering=False, debug=False,
                   num_devices=NCORES)
    bacc_mod = sys.modules["concourse.bacc"]
    bacc_mod.get_activation_tables = _patched_get_tables

    d_xa = nc.dram_tensor("xa", [128, T, KC, N], bf16, kind="ExternalInput").ap()
    d_w1 = nc.dram_tensor("w1a", [128, L, KC, 128], bf16, kind="ExternalInput").ap()
    d_rw = nc.dram_tensor("rwa", [128, KC, 64], bf16, kind="ExternalInput").ap()
    d_w2 = nc.dram_tensor("w2a", [128, NPAIR, 2, H2], bf16, kind="ExternalInput").ap()
    d_w3 = nc.dram_tensor("w3p", [128, NPAIR, 32], bf16, kind="ExternalInput").ap()
    d_a64 = nc.dram_tensor("a64", [128, L], bf16, kind="ExternalInput").ap()
    d_afl = nc.dram_tensor("afull", [128, 1024], bf16, kind="ExternalInput").ap()
    d_r = nc.dram_tensor("rsel", [128, OUT], bf16, kind="ExternalInput").ap()
    d_b3 = nc.dram_tensor("b3tp", [128, 2, OUT], bf16, kind="ExternalInput").ap()
    d_b1 = nc.dram_tensor("b1a", [128, L], f32, kind="ExternalInput").ap()
    d_b2 = nc.dram_tensor("b2a", [128, NPAIR], f32, kind="ExternalInput").ap()
    d_rbp = nc.dram_tensor("rbp", [128, 1], f32, kind="ExternalInput").ap()
    d_rbn = nc.dram_tensor("rbn", [128, 1], f32, kind="ExternalInput").ap()
    d_out = nc.dram_tensor("outT", [OUT, BC], f32, kind="ExternalOutput").ap()

    with tile.TileContext(nc) as tc:
        with tc.tile_pool(name="const", bufs=1) as cpool, \
             tc.tile_pool(name="route", bufs=1) as rpool, \
             tc.tile_pool(name="h1sb", bufs=10) as h1pool, \
             tc.tile_pool(name="h2sb", bufs=18) as h2pool, \
             tc.tile_pool(name="pasb", bufs=4) as papool, \
             tc.tile_pool(name="prod", bufs=5) as prpool, \
             tc.tile_pool(name="ps_h1", bufs=2, space="PSUM") as ps_h1, \
             tc.tile_pool(name="ps_w", bufs=4, space="PSUM") as ps_w:

            # ---- constants into SBUF, in byte-arrival order on the sync
            # queue: rwa (warmup/z) -> x t0 -> first W1 pair -> x t1 ->
            # rest of W1 g0 -> L2/L3/mix consts -> W1 g1..g7.
            rwa = cpool.tile([128, KC, 64], bf16)
            nc.sync.dma_start(rwa[:], d_rw)
            xa = cpool.tile([128, T, KC, N], bf16)
            nc.sync.dma_start(xa[:, 0], d_xa[:, 0])
            w1a = cpool.tile([128, L, KC, 128], bf16)
            nc.sync.dma_start(w1a[:, 0:2], d_w1[:, 0:2])
            nc.sync.dma_start(xa[:, 1], d_xa[:, 1])
            nc.sync.dma_start(w1a[:, 2:4], d_w1[:, 2:4])
            nc.sync.dma_start(w1a[:, 4:8], d_w1[:, 4:8])
            w2a = cpool.tile([128, NPAIR, 2, H2], bf16)
            nc.sync.dma_start(w2a[:], d_w2)
            w3p = cpool.tile([128, NPAIR, 32], bf16)
            nc.sync.dma_start(w3p[:], d_w3)
            a64 = cpool.tile([128, L], bf16)
            nc.sync.dma_start(a64[:], d_a64)
            afl = cpool.tile([128, 1024], bf16)
            nc.sync.dma_start(afl[:], d_afl)
            rsel = cpool.tile([128, OUT], bf16)
            nc.sync.dma_start(rsel[:], d_r)
            b3tp = cpool.tile([128, 2, OUT], bf16)
            nc.sync.dma_start(b3tp[:], d_b3)
            for g in range(1, NG):
                nc.sync.dma_start(w1a[:, 8 * g:8 * (g + 1)],
                                  d_w1[:, 8 * g:8 * (g + 1)])
            rbp = cpool.tile([128, 1], f32)
            nc.scalar.dma_start(rbp[:], d_rbp)
            rbn = cpool.tile([128, 1], f32)
            nc.scalar.dma_start(rbn[:], d_rbn)
            b1a = cpool.tile([128, L], f32)
            nc.scalar.dma_start(b1a[:], d_b1)
            b2a = cpool.tile([128, NPAIR], f32)
            nc.scalar.dma_start(b2a[:], d_b2)

            # ---- PE warm-up: dummy matmuls on an uninitialized SBUF tile
            # (no DMA dependency, so they issue right after the engine
            # preamble) keep the HAM activity window busy while x/W1
            # stream in, so real matmuls run at 2.4 GHz from the start.
            warm = cpool.tile([128, 256], bf16, name="warm")
            nc.gpsimd.memset(warm[:], 0.0)
            scratch = ps_w.tile([128, N], f32, tag="w", name="scratch")
            for w in range(20):
                nc.tensor.matmul(scratch[0:32, 0:256], warm[:, 0:32],
                                 warm[:, :], start=True, stop=True,
                                 tile_position=(0, 0))

            # ---- routing logits, packed [128, N]: rows 64t+32h hold the
            # (t, half-h) 32-node slice.  Serial per-(t,h) accumulation
            # chains (no interleaved start groups on a shared bank).
            z_ps = ps_w.tile([128, N], f32, tag="w", name="z_ps")
            for t in range(T):
                for c in range(KC):
                    nc.tensor.matmul(
                        z_ps[64 * t:64 * t + 64, :], rwa[:, c, :],
                        xa[:, t, c, :],
                        start=(c == 0), stop=(c == KC - 1),
                        tile_position=(0, 64 * t))

            def emit_routing():
                # routing ACT chain on the packed z (both t at once):
                #   ln s     = -(relu(-z') + ln(1 + exp(-|z'|)))
                #   ln (1-s) = -(relu( z') + ln(1 + exp(-|z'|)))
                # Emitted AFTER block 0's L1/evacuation ops so the (strict
                # FIFO) ACT/DVE queues don't head-of-line block the h1
                # evacuations behind this z-dependent chain.
                az = rpool.tile([128, N], f32, tag="az")
                nc.scalar.activation(az[:], z_ps[:], AF.Abs, bias=rbp[:],
                                     scale=1.0)
                eq = rpool.tile([128, N], f32, tag="eq")
                nc.scalar.activation(eq[:], az[:], AF.Exp, scale=-1.0)
                rzp = rpool.tile([128, N], f32, tag="rzp")
                nc.scalar.activation(rzp[:], z_ps[:], AF.Relu, bias=rbp[:],
                                     scale=1.0)
                rzn = rpool.tile([128, N], f32, tag="rzn")
                nc.scalar.activation(rzn[:], z_ps[:], AF.Relu, bias=rbn[:],
                                     scale=-1.0)
                nc.scalar.activation(eq[:], eq[:], AF.Ln, bias=1.0, scale=1.0)
                for t in range(T):
                    lo = 64 * t
                    s_t = cpool.tile([128, N], bf16, name=f"s{t}")
                    nc.vector.scalar_tensor_tensor(
                        s_t[0:64, :], rzn[lo:lo + 64, :], -1.0,
                        eq[lo:lo + 64, :], op0=ALU.mult, op1=ALU.subtract)
                    nc.vector.scalar_tensor_tensor(
                        s_t[64:128, :], rzp[lo:lo + 64, :], -1.0,
                        eq[lo:lo + 64, :], op0=ALU.mult, op1=ALU.subtract)
                    s_tiles.append(s_t)

            s_tiles = []
            h1sb = {}
            h2sb = {}
            pa_sb = {}
            prod_sb = {}
            out_sb = None
            ptx = None

            def emit_afl(g):
                """p values for group g, pre-broadcast (128-mode MMs)."""
                for t in range(T):
                    pa_ps = ps_w.tile([128, N], f32, tag="w",
                                      name=f"pa_ps{g}_{t}")
                    nc.tensor.matmul(pa_ps[:], afl[:, 128 * g:128 * (g + 1)],
                                     s_tiles[t][:], start=True, stop=True,
                                     tile_position=(0, 0))
                    pa_t = papool.tile([128, N], bf16, tag="pa",
                                       name=f"pa{g}_{t}")
                    nc.scalar.activation(pa_t[:], pa_ps[:], AF.Exp, scale=1.0)
                    pa_sb[(g, t)] = pa_t

            def emit_l3_prod(g, t):
                """pred for all 4 pairs of group g (one 4-tile burst) and
                the p-weighted product (DVE)."""
                pred_ps = ps_w.tile([128, N], f32, tag="w",
                                    name=f"pred{g}_{t}")
                for jj in range(4):
                    s_ = 4 * g + jj
                    nc.tensor.matmul(pred_ps[32 * jj:32 * jj + 32, :],
                                     w3p[:, s_, :], h2sb[(s_, t)][:],
                                     start=True, stop=True,
                                     tile_position=(0, 32 * jj))
                pr = prpool.tile([128, N], bf16, tag="pr", name=f"pr{g}_{t}")
                nc.vector.tensor_mul(pr[:], pred_ps[:], pa_sb[(g, t)][:])
                prod_sb[(g, t)] = pr

            def emit_r(g):
                """out += rsel^T prod for both batch tiles (one burst)."""
                r_ps = ps_w.tile([128, N], f32, tag="w", name=f"r_ps{g}")
                nc.tensor.matmul(r_ps[0:8, :], rsel[:], prod_sb[(g, 0)][:],
                                 start=True, stop=True, tile_position=(0, 0))
                nc.tensor.matmul(r_ps[32:40, :], rsel[:], prod_sb[(g, 1)][:],
                                 start=True, stop=True, tile_position=(0, 32))
                nc.vector.tensor_add(out_sb[:], out_sb[:], r_ps[0:64, :])

            # ---- main loop over 8-leaf groups: one 128-mode window (L1,
            # afl) + one 32-mode window (L2, deferred L3/prod/r) per block.
            for g in range(NG):
                # -------- 128-mode window --------
                # afl for the previous group leads the window: its pa
                # exp()s complete during the long L1 stretch, so the
                # 32-mode window's pred/r allocations never wait on them.
                if g >= 1:
                    emit_afl(g - 1)
                for jj in range(4):
                    s_ = 4 * g + jj
                    for e in range(2):
                        leaf = 2 * s_ + e
                        h1_ps = ps_h1.tile([128, T, N], f32, tag="h1",
                                           name=f"h1ps{leaf}")
                        # chunk-outer order: consecutive matmuls (t0, t1)
                        # share the same stationary tile, halving the
                        # weight-load traffic the PE has to hide.
                        for c in range(KC):
                            for t in range(T):
                                nc.tensor.matmul(
                                    h1_ps[:, t, :], w1a[:, leaf, c, :],
                                    xa[:, t, c, :],
                                    start=(c == 0), stop=(c == KC - 1),
                                    tile_position=(0, 0))
                        h1_t = h1pool.tile([128, T, N], bf16, tag="h1s",
                                           name=f"h1s{leaf}")
                        if leaf % 2 == 0:
                            nc.scalar.activation(h1_t[:], h1_ps[:], AF.Relu,
                                                 bias=b1a[:, leaf:leaf + 1],
                                                 scale=1.0)
                        else:
                            nc.vector.tensor_scalar(
                                h1_t[:], h1_ps[:], b1a[:, leaf:leaf + 1], 0.0,
                                op0=ALU.add, op1=ALU.max)
                        h1sb[leaf] = h1_t
                    if g == NG - 1 and jj == 0:
                        emit_afl(NG - 1)
                    if g == 0 and jj == 1:
                        emit_routing()

                # -------- 32-mode window --------
                if g == 1:
                    # leaf-arrival probabilities pt (packed t0|t1) and the
                    # b3 mixture term, all as 32-col-tile bursts.
                    pt_ps = ps_w.tile([128, N], f32, tag="w", name="pt_ps")
                    for t in range(T):
                        for h in range(2):
                            nc.tensor.matmul(
                                pt_ps[64 * t + 32 * h:64 * t + 32 * h + 32, :],
                                a64[:, 32 * h:32 * h + 32], s_tiles[t][:],
                                start=True, stop=True,
                                tile_position=(0, 64 * t + 32 * h))
                    ptx = cpool.tile([128, N], bf16, name="ptx")
                    nc.scalar.activation(ptx[:], pt_ps[:], AF.Exp, scale=1.0)
                    rb_ps = ps_w.tile([128, N], f32, tag="w", name="rb_ps")
                    nc.tensor.matmul(rb_ps[0:8, :], b3tp[:, 0, :], ptx[:],
                                     start=True, stop=True,
                                     tile_position=(0, 0))
                    nc.tensor.matmul(rb_ps[32:40, :], b3tp[:, 1, :], ptx[:],
                                     start=True, stop=True,
                                     tile_position=(0, 32))
                    out_sb = cpool.tile([64, N], f32, name="out_sb")
                    nc.scalar.activation(out_sb[:], rb_ps[0:64, :], AF.Copy,
                                         scale=1.0)
                # deferred L3 leads the window: its pred allocation's
                # bank-reuse gate (a previous-block evacuation) is long
                # done, and its prod (DVE) lands before the first h2
                # bank-reuse gate comes due.
                if g >= 1:
                    emit_l3_prod(g - 1, 0)
                for jj in range(4):
                    s_ = 4 * g + jj
                    for t in range(T):
                        h2_ps = ps_w.tile([128, N], f32, tag="w",
                                           name=f"h2ps{s_}_{t}")
                        for e in range(2):
                            for h in range(2):
                                base = 64 * e + 32 * h
                                nc.tensor.matmul(
                                    h2_ps[base:base + 32, :],
                                    w2a[:, s_, e, 32 * h:32 * h + 32],
                                    h1sb[2 * s_ + e][:, t, :],
                                    start=True, stop=True,
                                    tile_position=(0, base))
                        h2_t = h2pool.tile([128, N], bf16, tag="h2s",
                                           name=f"h2s{s_}_{t}")
                        # h2-evac engine split: consecutive bank frees
                        # alternate engines; DVE (which also runs the
                        # window's prod/add ops) gets 3 of 8.
                        if t == 0 or jj == 3:
                            nc.scalar.activation(h2_t[:], h2_ps[:], AF.Relu,
                                                 bias=b2a[:, s_:s_ + 1],
                                                 scale=1.0)
                        else:
                            nc.vector.tensor_scalar(
                                h2_t[:], h2_ps[:], b2a[:, s_:s_ + 1], 0.0,
                                op0=ALU.add, op1=ALU.max)
                        h2sb[(s_, t)] = h2_t
                    # deferred mixing work, kept inside the 32-mode window
                    if jj == 0 and g >= 1:
                        emit_l3_prod(g - 1, 1)
                    if jj == 1 and g >= 2:
                        emit_r(g - 2)

            # ---- tail: last groups' L3/prod/r, then write out ----
            emit_l3_prod(NG - 1, 0)
            emit_l3_prod(NG - 1, 1)
            emit_r(NG - 2)
            emit_r(NG - 1)
            nc.sync.dma_start(d_out[:, 0:N], out_sb[0:8, :])
            nc.sync.dma_start(d_out[:, N:2 * N], out_sb[32:40, :])

    try:
        nc.compile()
    finally:
        bacc_mod.get_activation_tables = _orig_get_tables
    return nc


def pack_shared(router_W, router_b, W1, b1, W2, b2, W3, b3):
    """Host-side packing of replicated parameters into SBUF-friendly layouts."""
    f = np.float32
    router_W = np.asarray(router_W, f)
    router_b = np.asarray(router_b, f)
    W1 = np.asarray(W1, f)
    b1 = np.asarray(b1, f)
    W2 = np.asarray(W2, f)
    b2 = np.asarray(b2, f)
    W3 = np.asarray(W3, f)
    b3 = np.asarray(b3, f)

    w1a = np.ascontiguousarray(
        W1.reshape(L, KC, 128, H1).transpose(2, 0, 1, 3)).astype(BF)
    rwa = np.zeros((128, KC, 64), f)
    rwa[:, :, 0:NI] = router_W.T.reshape(KC, 128, NI).transpose(1, 0, 2)
    w2a = np.ascontiguousarray(
        W2.reshape(NPAIR, 2, H1, H2).transpose(2, 0, 1, 3)).astype(BF)

    w3p = np.zeros((NPAIR, 128, 32), f)
    for s in range(NPAIR):
        w3p[s, 0:64, 0:8] = W3[2 * s]
        w3p[s, 64:128, 8:16] = W3[2 * s + 1]
    w3p = np.ascontiguousarray(w3p.transpose(1, 0, 2)).astype(BF)

    a64 = np.zeros((128, L), f)
    for leaf in range(L):
        for row in _leaf_path_rows(leaf):
            a64[row, leaf] += 1.0

    afull = np.zeros((128, 1024), f)
    for g in range(NG):
        for jj in range(4):
            s = 4 * g + jj
            for m in range(16):
                leaf = 2 * s + (m >= 8)
                col = 128 * g + 32 * jj + m
                for row in _leaf_path_rows(leaf):
                    afull[row, col] += 1.0

    rsel = np.zeros((128, OUT), f)
    for kk in range(128):
        m = kk % 32
        if m < 8:
            rsel[kk, m] = 1.0
        elif m < 16:
            rsel[kk, m - 8] = 1.0

    b3tp = np.zeros((128, 2, OUT), f)
    b3tp[0:64, 0, :] = b3
    b3tp[64:128, 1, :] = b3

    rb_pad = np.concatenate([router_b, [0.0]]).astype(f)
    rbp = np.concatenate([rb_pad, rb_pad])[:, None]
    rbn = np.concatenate([-rb_pad, -rb_pad])[:, None]

    return {
        "w1a": w1a,
        "rwa": rwa.astype(BF),
        "w2a": w2a,
        "w3p": w3p,
        "a64": a64.astype(BF),
        "afull": afull.astype(BF),
        "rsel": rsel.astype(BF),
        "b3tp": b3tp.astype(BF),
        "b1a": np.ascontiguousarray(b1.T),
        "b2a": np.ascontiguousarray(b2.reshape(NPAIR, 128).T),
        "rbp": rbp,
        "rbn": rbn,
    }


def pack_x_core(x_core):
    """[1024, 512] slice -> [128, T, KC, 512] tile-major transposed layout."""
    xc = np.asarray(x_core, np.float32)
    parts = []
    for t in range(T):
        parts.append(xc[N * t:N * (t + 1)].T.reshape(KC, 128, N))
    stacked = np.stack(parts, axis=0)            # [T, KC, 128, N]
    return np.ascontiguousarray(stacked.transpose(2, 0, 1, 3)).astype(BF)


_NC_CACHE = {}


def _get_nc():
    if "nc" not in _NC_CACHE:
        _NC_CACHE["nc"] = build_nc()
    return _NC_CACHE["nc"]


def kernel(**inputs):
    x = np.asarray(inputs["x"], np.float32)
    shared = pack_shared(inputs["router_W"], inputs["router_b"],
                         inputs["W1"], inputs["b1"], inputs["W2"],
                         inputs["b2"], inputs["W3"], inputs["b3"])
    in_maps = []
    for i in range(NCORES):
        m = dict(shared)
        m["xa"] = pack_x_core(x[BC * i:BC * (i + 1)])
        in_maps.append(m)
    nc = _get_nc()
    res = run_bass_kernel_spmd(nc, in_maps, core_ids=list(range(NCORES)))
    out = np.concatenate([r["outT"].T for r in res.results], axis=0)
    return np.ascontiguousarray(out, np.float32)


# revision 19
# speedup vs baseline: 1.0425x; 1.0425x over previous
"""Trainium2 Bass kernel for NeuralDecisionTree (soft decision tree MoE).

Strategy: data-parallel over batch across 8 NeuronCores (1024 rows/core),
weights replicated, all matmul operands in bfloat16 (fp32 PSUM accumulate).
bf16 (vs the earlier fp32r version) lets LDWEIGHTS pipeline ahead of the
streaming matmuls (fp32r fuses the 4-byte weight load serially into every
matmul: ~60ns/MM extra), halves HBM traffic, and permits PSUM writes at
non-zero partition bases, which unlocks column-tiled (tile_position)
concurrent matmuls for every stationary-width<128 GEMM.

Per-core dataflow (activations kept in [feature, batch] layout):
  z      = router_W @ x^T     16 MMs in 32-col-tile mode, packed [128, 512]
                              (t0 rows 0-63, t1 rows 64-127)
  S_t    = [ln s; ln(1-s)]    routing ACT chain on the packed z, 2 DVE builds
  log p  = A @ S -> exp       a64 (pt, 4-tile burst) + per-group afl
                              pre-broadcast selection matmuls (128-mode)
  L1:    h1[l] = relu(W1_l^T x^T + b1)  4 K-chunk MMs per (leaf, t) into a
         2-bank PSUM tile (t0|t1), one paired [128,2,512] evacuation
  L2:    per leaf pair: 4 concurrent 32-col-tile MMs -> [128,512] PSUM
  L3:    per group: 4 pairs as 4 concurrent 32-col tiles -> [128,512] pred
  mix:   prod = pred * p (DVE), out += rsel^T prod (32-mode MM bursts),
         b3 term via padded b3 stationaries against packed exp(pt).

The kernel alternates one 128-mode window (L1 + afl) and one 32-mode
window (L2 + deferred L3/prod/r of earlier groups) per 8-leaf block, so
the PE pays only 2 tiling-mode switches per block.  A run of dummy
matmuls on the (tiny, first-DMA'd) router weights warms the PE HAM clock
gate during the initial x/W1 DMA window.
"""

import sys

import numpy as np
import ml_dtypes

if "/opt/trn_rl_repo" not in sys.path:
    sys.path.insert(0, "/opt/trn_rl_repo")

import concourse.bass as bass  # noqa: F401  (import keeps parity with env)
import concourse.hw_specs as hw_specs
import concourse.tile as tile
from concourse import bacc, mybir
from concourse.bass_utils import run_bass_kernel_spmd

_ONE_TABLE = "natural_log_exp_and_others"
_orig_get_tables = hw_specs.get_activation_tables


def _patched_get_tables(module_arch):
    """Confine activation-table choice to one set that covers every ACT
    func this kernel uses (exp/ln/relu/abs/copy/identity), so the greedy
    per-instruction table picker never ping-pongs between sets."""
    tables = dict(_orig_get_tables(module_arch))
    keep = tables[_ONE_TABLE]
    return {k: (v if k == _ONE_TABLE else (v & set()) or set())
            if k != _ONE_TABLE else keep for k, v in tables.items()}


f32 = mybir.dt.float32
bf16 = mybir.dt.bfloat16
AF = mybir.ActivationFunctionType
ALU = mybir.AluOpType
BF = ml_dtypes.bfloat16

# Problem shape (hardcoded; harness contract)
B = 8192
D = 512
H1 = 128
H2 = 64
OUT = 8
L = 64
NI = 63
NCORES = 8
BC = B // NCORES        # 1024 rows per core
N = 512                 # batch tile (matmul free dim / PSUM bank)
T = BC // N             # 2 batch tiles per core
KC = D // 128           # 4 contraction chunks for the input dim
NPAIR = L // 2          # 32 leaf pairs
NG = 8                  # 8-leaf groups


def _leaf_path_rows(leaf):
    """Rows of the [128] log-sigmoid stack contributing to log p(leaf).

    Row n (n<63) holds ln d_n; row 64+n holds ln(1-d_n); rows 63 and 127
    are zero pads.  Mirrors the reference's level-wise p interleave.
    """
    rows = []
    for k in range(6):
        prefix = leaf >> (6 - k)
        node = (2 ** k - 1) + prefix
        bit = (leaf >> (5 - k)) & 1
        rows.append(node + 64 * bit)
    return rows


def build_nc():
    nc = bacc.Bacc("TRN2", target_bir_lowering=False, debug=False,
                   num_devices=NCORES)
    bacc_mod = sys.modules["concourse.bacc"]
    bacc_mod.get_activation_tables = _patched_get_tables

    d_xa = nc.dram_tensor("xa", [128, T, KC, N], bf16, kind="ExternalInput").ap()
    d_w1 = nc.dram_tensor("w1a", [128, L, KC, 128], bf16, kind="ExternalInput").ap()
    d_rw = nc.dram_tensor("rwa", [128, KC, 64], bf16, kind="ExternalInput").ap()
    d_w2 = nc.dram_tensor("w2a", [128, NPAIR, 2, H2], bf16, kind="ExternalInput").ap()
    d_w3 = nc.dram_tensor("w3p", [128, NPAIR, 32], bf16, kind="ExternalInput").ap()
    d_a64 = nc.dram_tensor("a64", [128, L], bf16, kind="ExternalInput").ap()
    d_afl = nc.dram_tensor("afull", [128, 1024], bf16, kind="ExternalInput").ap()
    d_r = nc.dram_tensor("rsel", [128, OUT], bf16, kind="ExternalInput").ap()
    d_b3 = nc.dram_tensor("b3tp", [128, 2, OUT], bf16, kind="ExternalInput").ap()
    d_b1 = nc.dram_tensor("b1a", [128, L], f32, kind="ExternalInput").ap()
    d_b2 = nc.dram_tensor("b2a", [128, NPAIR], f32, kind="ExternalInput").ap()
    d_rbp = nc.dram_tensor("rbp", [128, 1], f32, kind="ExternalInput").ap()
    d_rbn = nc.dram_tensor("rbn", [128, 1], f32, kind="ExternalInput").ap()
    d_out = nc.dram_tensor("outT", [OUT, BC], f32, kind="ExternalOutput").ap()

    with tile.TileContext(nc) as tc:
        with tc.tile_pool(name="const", bufs=1) as cpool, \
             tc.tile_pool(name="route", bufs=1) as rpool, \
             tc.tile_pool(name="h1sb", bufs=10) as h1pool, \
             tc.tile_pool(name="h2sb", bufs=18) as h2pool, \
             tc.tile_pool(name="pasb", bufs=4) as papool, \
             tc.tile_pool(name="prod", bufs=5) as prpool, \
             tc.tile_pool(name="ps_h1", bufs=2, space="PSUM") as ps_h1, \
             tc.tile_pool(name="ps_w", bufs=4, space="PSUM") as ps_w:

            # ---- constants into SBUF, in byte-arrival order on the sync
            # queue: rwa (warmup/z) -> x t0 -> first W1 pair -> x t1 ->
            # rest of W1 g0 -> L2/L3/mix consts -> W1 g1..g7.
            rwa = cpool.tile([128, KC, 64], bf16)
            nc.sync.dma_start(rwa[:], d_rw)
            xa = cpool.tile([128, T, KC, N], bf16)
            nc.sync.dma_start(xa[:, 0], d_xa[:, 0])
            w1a = cpool.tile([128, L, KC, 128], bf16)
            nc.sync.dma_start(w1a[:, 0:2], d_w1[:, 0:2])
            nc.sync.dma_start(xa[:, 1], d_xa[:, 1])
            nc.sync.dma_start(w1a[:, 2:4], d_w1[:, 2:4])
            nc.sync.dma_start(w1a[:, 4:8], d_w1[:, 4:8])
            w2a = cpool.tile([128, NPAIR, 2, H2], bf16)
            nc.sync.dma_start(w2a[:], d_w2)
            w3p = cpool.tile([128, NPAIR, 32], bf16)
            nc.sync.dma_start(w3p[:], d_w3)
            a64 = cpool.tile([128, L], bf16)
            nc.sync.dma_start(a64[:], d_a64)
            afl = cpool.tile([128, 1024], bf16)
            nc.sync.dma_start(afl[:], d_afl)
            rsel = cpool.tile([128, OUT], bf16)
            nc.sync.dma_start(rsel[:], d_r)
            b3tp = cpool.tile([128, 2, OUT], bf16)
            nc.sync.dma_start(b3tp[:], d_b3)
            for g in range(1, NG):
                nc.sync.dma_start(w1a[:, 8 * g:8 * (g + 1)],
                                  d_w1[:, 8 * g:8 * (g + 1)])
            rbp = cpool.tile([128, 1], f32)
            nc.scalar.dma_start(rbp[:], d_rbp)
            rbn = cpool.tile([128, 1], f32)
            nc.scalar.dma_start(rbn[:], d_rbn)
            b1a = cpool.tile([128, L], f32)
            nc.scalar.dma_start(b1a[:], d_b1)
            b2a = cpool.tile([128, NPAIR], f32)
            nc.scalar.dma_start(b2a[:], d_b2)

            # ---- PE warm-up: dummy matmuls on an uninitialized SBUF tile
            # (no DMA dependency, so they issue right after the engine
            # preamble) keep the HAM activity window busy while x/W1
            # stream in, so real matmuls run at 2.4 GHz from the start.
            warm = cpool.tile([128, 256], bf16, name="warm")
            nc.gpsimd.memset(warm[:], 0.0)
            scratch = ps_w.tile([128, N], f32, tag="w", name="scratch")
            for w in range(28):
                nc.tensor.matmul(scratch[0:32, 0:256], warm[:, 0:32],
                                 warm[:, :], start=True, stop=True,
                                 tile_position=(0, 0))

            # ---- routing logits, packed [128, N]: rows 64t+32h hold the
            # (t, half-h) 32-node slice.  Serial per-(t,h) accumulation
            # chains (no interleaved start groups on a shared bank).
            z_ps = ps_w.tile([128, N], f32, tag="w", name="z_ps")
            for t in range(T):
                for c in range(KC):
                    nc.tensor.matmul(
                        z_ps[64 * t:64 * t + 64, :], rwa[:, c, :],
                        xa[:, t, c, :],
                        start=(c == 0), stop=(c == KC - 1),
                        tile_position=(0, 64 * t))

            def emit_routing():
                # routing ACT chain on the packed z (both t at once):
                #   ln s     = -(relu(-z') + ln(1 + exp(-|z'|)))
                #   ln (1-s) = -(relu( z') + ln(1 + exp(-|z'|)))
                # Emitted AFTER block 0's L1/evacuation ops so the (strict
                # FIFO) ACT/DVE queues don't head-of-line block the h1
                # evacuations behind this z-dependent chain.
                az = rpool.tile([128, N], f32, tag="az")
                nc.scalar.activation(az[:], z_ps[:], AF.Abs, bias=rbp[:],
                                     scale=1.0)
                eq = rpool.tile([128, N], f32, tag="eq")
                nc.scalar.activation(eq[:], az[:], AF.Exp, scale=-1.0)
                rzp = rpool.tile([128, N], f32, tag="rzp")
                nc.scalar.activation(rzp[:], z_ps[:], AF.Relu, bias=rbp[:],
                                     scale=1.0)
                rzn = rpool.tile([128, N], f32, tag="rzn")
                nc.scalar.activation(rzn[:], z_ps[:], AF.Relu, bias=rbn[:],
                                     scale=-1.0)
                nc.scalar.activation(eq[:], eq[:], AF.Ln, bias=1.0, scale=1.0)
                for t in range(T):
                    lo = 64 * t
                    s_t = cpool.tile([128, N], bf16, name=f"s{t}")
                    nc.vector.scalar_tensor_tensor(
                        s_t[0:64, :], rzn[lo:lo + 64, :], -1.0,
                        eq[lo:lo + 64, :], op0=ALU.mult, op1=ALU.subtract)
                    nc.vector.scalar_tensor_tensor(
                        s_t[64:128, :], rzp[lo:lo + 64, :], -1.0,
                        eq[lo:lo + 64, :], op0=ALU.mult, op1=ALU.subtract)
                    s_tiles.append(s_t)

            s_tiles = []
            h1sb = {}
            h2sb = {}
            pa_sb = {}
            prod_sb = {}
            out_sb = None
            ptx = None

            def emit_afl(g):
                """p values for group g, pre-broadcast (128-mode MMs)."""
                for t in range(T):
                    pa_ps = ps_w.tile([128, N], f32, tag="w",
                                      name=f"pa_ps{g}_{t}")
                    nc.tensor.matmul(pa_ps[:], afl[:, 128 * g:128 * (g + 1)],
                                     s_tiles[t][:], start=True, stop=True,
                                     tile_position=(0, 0))
                    pa_t = papool.tile([128, N], bf16, tag="pa",
                                       name=f"pa{g}_{t}")
                    nc.scalar.activation(pa_t[:], pa_ps[:], AF.Exp, scale=1.0)
                    pa_sb[(g, t)] = pa_t

            def emit_l3_prod(g, t):
                """pred for all 4 pairs of group g (one 4-tile burst) and
                the p-weighted product (DVE)."""
                pred_ps = ps_w.tile([128, N], f32, tag="w",
                                    name=f"pred{g}_{t}")
                for jj in range(4):
                    s_ = 4 * g + jj
                    nc.tensor.matmul(pred_ps[32 * jj:32 * jj + 32, :],
                                     w3p[:, s_, :], h2sb[(s_, t)][:],
                                     start=True, stop=True,
                                     tile_position=(0, 32 * jj))
                pr = prpool.tile([128, N], bf16, tag="pr", name=f"pr{g}_{t}")
                nc.vector.tensor_mul(pr[:], pred_ps[:], pa_sb[(g, t)][:])
                prod_sb[(g, t)] = pr

            def emit_r(g):
                """out += rsel^T prod for both batch tiles (one burst)."""
                r_ps = ps_w.tile([128, N], f32, tag="w", name=f"r_ps{g}")
                nc.tensor.matmul(r_ps[0:8, :], rsel[:], prod_sb[(g, 0)][:],
                                 start=True, stop=True, tile_position=(0, 0))
                nc.tensor.matmul(r_ps[32:40, :], rsel[:], prod_sb[(g, 1)][:],
                                 start=True, stop=True, tile_position=(0, 32))
                nc.vector.tensor_add(out_sb[:], out_sb[:], r_ps[0:64, :])

            # ---- main loop over 8-leaf groups: one 128-mode window (L1,
            # afl) + one 32-mode window (L2, deferred L3/prod/r) per block.
            for g in range(NG):
                # -------- 128-mode window --------
                # afl for the previous group leads the window: its pa
                # exp()s complete during the long L1 stretch, so the
                # 32-mode window's pred/r allocations never wait on them.
                if g >= 1:
                    emit_afl(g - 1)
                for jj in range(4):
                    s_ = 4 * g + jj
                    for e in range(2):
                        leaf = 2 * s_ + e
                        h1_ps = ps_h1.tile([128, T, N], f32, tag="h1",
                                           name=f"h1ps{leaf}")
                        # chunk-outer order: consecutive matmuls (t0, t1)
                        # share the same stationary tile, halving the
                        # weight-load traffic the PE has to hide.
                        for c in range(KC):
                            for t in range(T):
                                nc.tensor.matmul(
                                    h1_ps[:, t, :], w1a[:, leaf, c, :],
                                    xa[:, t, c, :],
                                    start=(c == 0), stop=(c == KC - 1),
                                    tile_position=(0, 0))
                        h1_t = h1pool.tile([128, T, N], bf16, tag="h1s",
                                           name=f"h1s{leaf}")
                        if leaf % 2 == 0:
                            nc.scalar.activation(h1_t[:], h1_ps[:], AF.Relu,
                                                 bias=b1a[:, leaf:leaf + 1],
                                                 scale=1.0)
                        else:
                            nc.vector.tensor_scalar(
                                h1_t[:], h1_ps[:], b1a[:, leaf:leaf + 1], 0.0,
                                op0=ALU.add, op1=ALU.max)
                        h1sb[leaf] = h1_t
                    if g == NG - 1 and jj == 0:
                        emit_afl(NG - 1)
                    if g == 0 and jj == 1:
                        emit_routing()

                # -------- 32-mode window --------
                if g == 1:
                    # leaf-arrival probabilities pt (packed t0|t1) and the
                    # b3 mixture term, all as 32-col-tile bursts.
                    pt_ps = ps_w.tile([128, N], f32, tag="w", name="pt_ps")
                    for t in range(T):
                        for h in range(2):
                            nc.tensor.matmul(
                                pt_ps[64 * t + 32 * h:64 * t + 32 * h + 32, :],
                                a64[:, 32 * h:32 * h + 32], s_tiles[t][:],
                                start=True, stop=True,
                                tile_position=(0, 64 * t + 32 * h))
                    ptx = cpool.tile([128, N], bf16, name="ptx")
                    nc.scalar.activation(ptx[:], pt_ps[:], AF.Exp, scale=1.0)
                    rb_ps = ps_w.tile([128, N], f32, tag="w", name="rb_ps")
                    nc.tensor.matmul(rb_ps[0:8, :], b3tp[:, 0, :], ptx[:],
                                     start=True, stop=True,
                                     tile_position=(0, 0))
                    nc.tensor.matmul(rb_ps[32:40, :], b3tp[:, 1, :], ptx[:],
                                     start=True, stop=True,
                                     tile_position=(0, 32))
                    out_sb = cpool.tile([64, N], f32, name="out_sb")
                    nc.scalar.activation(out_sb[:], rb_ps[0:64, :], AF.Copy,
                                         scale=1.0)
                for jj in range(4):
                    s_ = 4 * g + jj
                    for t in range(T):
                        h2_ps = ps_w.tile([128, N], f32, tag="w",
                                           name=f"h2ps{s_}_{t}")
                        for e in range(2):
                            for h in range(2):
                                base = 64 * e + 32 * h
                                nc.tensor.matmul(
                                    h2_ps[base:base + 32, :],
                                    w2a[:, s_, e, 32 * h:32 * h + 32],
                                    h1sb[2 * s_ + e][:, t, :],
                                    start=True, stop=True,
                                    tile_position=(0, base))
                        h2_t = h2pool.tile([128, N], bf16, tag="h2s",
                                           name=f"h2s{s_}_{t}")
                        # window-evac balance: DVE also runs prod/add in
                        # the 32-mode window, so ACT takes 6 of 8 h2
                        # evacuations per window.
                        if not (t == 1 and jj % 2 == 1):
                            nc.scalar.activation(h2_t[:], h2_ps[:], AF.Relu,
                                                 bias=b2a[:, s_:s_ + 1],
                                                 scale=1.0)
                        else:
                            nc.vector.tensor_scalar(
                                h2_t[:], h2_ps[:], b2a[:, s_:s_ + 1], 0.0,
                                op0=ALU.add, op1=ALU.max)
                        h2sb[(s_, t)] = h2_t
                    # deferred mixing work, kept inside the 32-mode window
                    if jj == 0 and g >= 1:
                        emit_l3_prod(g - 1, 0)
                    if jj == 1 and g >= 1:
                        emit_l3_prod(g - 1, 1)
                    if jj == 2 and g >= 2:
                        emit_r(g - 2)

            # ---- tail: last groups' L3/prod/r, then write out ----
            emit_l3_prod(NG - 1, 0)
            emit_l3_prod(NG - 1, 1)
            emit_r(NG - 2)
            emit_r(NG - 1)
            nc.sync.dma_start(d_out[:, 0:N], out_sb[0:8, :])
            nc.sync.dma_start(d_out[:, N:2 * N], out_sb[32:40, :])

    try:
        nc.compile()
    finally:
        bacc_mod.get_activation_tables = _orig_get_tables
    return nc


def pack_shared(router_W, router_b, W1, b1, W2, b2, W3, b3):
    """Host-side packing of replicated parameters into SBUF-friendly layouts."""
    f = np.float32
    router_W = np.asarray(router_W, f)
    router_b = np.asarray(router_b, f)
    W1 = np.asarray(W1, f)
    b1 = np.asarray(b1, f)
    W2 = np.asarray(W2, f)
    b2 = np.asarray(b2, f)
    W3 = np.asarray(W3, f)
    b3 = np.asarray(b3, f)

    w1a = np.ascontiguousarray(
        W1.reshape(L, KC, 128, H1).transpose(2, 0, 1, 3)).astype(BF)
    rwa = np.zeros((128, KC, 64), f)
    rwa[:, :, 0:NI] = router_W.T.reshape(KC, 128, NI).transpose(1, 0, 2)
    w2a = np.ascontiguousarray(
        W2.reshape(NPAIR, 2, H1, H2).transpose(2, 0, 1, 3)).astype(BF)

    w3p = np.zeros((NPAIR, 128, 32), f)
    for s in range(NPAIR):
        w3p[s, 0:64, 0:8] = W3[2 * s]
        w3p[s, 64:128, 8:16] = W3[2 * s + 1]
    w3p = np.ascontiguousarray(w3p.transpose(1, 0, 2)).astype(BF)

    a64 = np.zeros((128, L), f)
    for leaf in range(L):
        for row in _leaf_path_rows(leaf):
            a64[row, leaf] += 1.0

    afull = np.zeros((128, 1024), f)
    for g in range(NG):
        for jj in range(4):
            s = 4 * g + jj
            for m in range(16):
                leaf = 2 * s + (m >= 8)
                col = 128 * g + 32 * jj + m
                for row in _leaf_path_rows(leaf):
                    afull[row, col] += 1.0

    rsel = np.zeros((128, OUT), f)
    for kk in range(128):
        m = kk % 32
        if m < 8:
            rsel[kk, m] = 1.0
        elif m < 16:
            rsel[kk, m - 8] = 1.0

    b3tp = np.zeros((128, 2, OUT), f)
    b3tp[0:64, 0, :] = b3
    b3tp[64:128, 1, :] = b3

    rb_pad = np.concatenate([router_b, [0.0]]).astype(f)
    rbp = np.concatenate([rb_pad, rb_pad])[:, None]
    rbn = np.concatenate([-rb_pad, -rb_pad])[:, None]

    return {
        "w1a": w1a,
        "rwa": rwa.astype(BF),
        "w2a": w2a,
        "w3p": w3p,
        "a64": a64.astype(BF),
        "afull": afull.astype(BF),
        "rsel": rsel.astype(BF),
        "b3tp": b3tp.astype(BF),
        "b1a": np.ascontiguousarray(b1.T),
        "b2a": np.ascontiguousarray(b2.reshape(NPAIR, 128).T),
        "rbp": rbp,
        "rbn": rbn,
    }


def pack_x_core(x_core):
    """[1024, 512] slice -> [128, T, KC, 512] tile-major transposed layout."""
    xc = np.asarray(x_core, np.float32)
    parts = []
    for t in range(T):
        parts.append(xc[N * t:N * (t + 1)].T.reshape(KC, 128, N))
    stacked = np.stack(parts, axis=0)            # [T, KC, 128, N]
    return np.ascontiguousarray(stacked.transpose(2, 0, 1, 3)).astype(BF)


_NC_CACHE = {}


def _get_nc():
    if "nc" not in _NC_CACHE:
        _NC_CACHE["nc"] = build_nc()
    return _NC_CACHE["nc"]


def kernel(**inputs):
    x = np.asarray(inputs["x"], np.float32)
    shared = pack_shared(inputs["router_W"], inputs["router_b"],
                         inputs["W1"], inputs["b1"], inputs["W2"],
                         inputs["b2"], inputs["W3"], inputs["b3"])
    in_maps = []
    for i in range(NCORES):
        m = dict(shared)
        m["xa"] = pack_x_core(x[BC * i:BC * (i + 1)])
        in_maps.append(m)
    nc = _get_nc()
    res = run_bass_kernel_spmd(nc, in_maps, core_ids=list(range(NCORES)))
    out = np.concatenate([r["outT"].T for r in res.results], axis=0)
    return np.ascontiguousarray(out, np.float32)
